# revision 11
# baseline (speedup 1.0000x reference)
"""Batched attention kernel for Trainium2, SPMD over 8 NeuronCores.

Computes, for inputs K, V, Q of shape [16, 2048, 256] (f32):
    A = softmax(Q @ K^T / sqrt(256), axis=-1)      # [16, 2048, 2048]
    R = concat(A @ V, Q, axis=-1)                  # [16, 2048, 512]
and returns (R, A), matching the reference.

Sharding: batch dim across the 8 cores (2 batches per core), fully local.

Per-core dataflow (per batch):
  prep: load K/Q/V tiles f32; K^T/Q^T built via PE transpose + DVE
        evict-cast to bf16; V cast to bf16; Q f32 DMA'd out to R[..., D:].
  main (16 q-tiles of 128 rows):
        S = Q@K^T into PSUM (bf16 matmuls, f32 accum, 2 chunk tiles)
        E_c = exp(S_c/16) via ScalarE with free row-sum accumulation
        A = E * (1/rowsum) on ScalarE (deferred one tile so the next
        tile's exp isn't stuck behind it in the ScalarE queue), DMA out
        E^T via PE transposes + DVE evicts (per chunk, so chunk 0
        transposes overlap chunk 1's exp)
        O = E^T.T @ V accumulated in PSUM, scaled by 1/rowsum, DMA to R
  Softmax max-subtraction is skipped: scores ~ N(0,1), no overflow risk.
  Emission order staggers prep with compute: K(0) prep first, then Q/V(0)
  and the next batch's prep interleave into the running main loop.
"""

import numpy as np

B, T, D = 16, 2048, 256
NCORES = 8
BPC = B // NCORES   # batches per core
NT = T // 128       # 16 row-tiles per sequence
ND = D // 128       # 2 contraction chunks

SCALE = 1.0 / float(np.sqrt(np.float32(D)))  # 1/16


def build_nc(
    n_schunks=2,          # S psum chunks per q-tile (each T//n_schunks wide)
    spsum_bufs=2,
    tpsum_bufs=3,
    opsum_bufs=1,
    e_bufs=6,             # chunk-granular E tiles
    et_bufs=2,
    a_bufs=3,
    anorm_engine="dve_bf16",  # "dve_bf16" | "scalar" | "vector" | "alternate"
    anorm_lag=1,          # tiles to defer the A-normalize + A DMA by
):
    from contextlib import ExitStack
    import concourse.bacc as bacc
    import concourse.tile as tile
    from concourse import mybir, masks

    f32 = mybir.dt.float32
    bf16 = mybir.dt.bfloat16
    AF = mybir.ActivationFunctionType

    CW = T // n_schunks            # chunk width in k
    NC_T = CW // 128               # k-subtiles per chunk
    assert CW % 512 == 0

    nc = bacc.Bacc(None, target_bir_lowering=False)
    Kd = nc.declare_dram_parameter("K", [BPC, T, D], f32, isOutput=False)
    Vd = nc.declare_dram_parameter("V", [BPC, T, D], f32, isOutput=False)
    Qd = nc.declare_dram_parameter("Q", [BPC, T, D], f32, isOutput=False)
    Rd = nc.declare_dram_parameter("R", [BPC, T, 2 * D], f32, isOutput=True)
    Ad = nc.declare_dram_parameter("A", [BPC, T, T], f32, isOutput=True)

    with tile.TileContext(nc) as tc, ExitStack() as ctx:
        singles = ctx.enter_context(tc.tile_pool(name="singles", bufs=1))
        batchp = ctx.enter_context(tc.tile_pool(name="batchp", bufs=2))
        loads = ctx.enter_context(tc.tile_pool(name="loads", bufs=6))
        epool = ctx.enter_context(tc.tile_pool(name="epool", bufs=e_bufs))
        etpool = ctx.enter_context(tc.tile_pool(name="etpool", bufs=et_bufs))
        apool = ctx.enter_context(tc.tile_pool(name="apool", bufs=a_bufs))
        rpool = ctx.enter_context(tc.tile_pool(name="rpool", bufs=3))
        small = ctx.enter_context(tc.tile_pool(name="small", bufs=8))
        spsum = ctx.enter_context(tc.tile_pool(name="spsum", bufs=spsum_bufs, space="PSUM"))
        opsum = ctx.enter_context(tc.tile_pool(name="opsum", bufs=opsum_bufs, space="PSUM"))
        tpsum = ctx.enter_context(tc.tile_pool(name="tpsum", bufs=tpsum_bufs, space="PSUM"))

        ident_f32 = singles.tile([128, 128], f32)
        masks.make_identity(nc, ident_f32)
        ident_bf16 = singles.tile([128, 128], bf16)
        masks.make_identity(nc, ident_bf16)

        batch_tiles = {}
        pending_anorm = []   # deferred (b, qt, Ecs, rinv) entries

        def prep_start(b):
            batch_tiles[b] = {
                "KT": batchp.tile([128, ND, NT, 128], bf16, tag="KT", name=f"KT{b}"),
                "QT": batchp.tile([128, ND, NT, 128], bf16, tag="QT", name=f"QT{b}"),
                "Vb": batchp.tile([128, NT, D], bf16, tag="Vb", name=f"Vb{b}"),
            }

        def prep_k(b, t):
            KT = batch_tiles[b]["KT"]
            ld = loads.tile([128, D], f32, tag="ld", name=f"kld{b}_{t}")
            nc.sync.dma_start(out=ld, in_=Kd[b, t * 128:(t + 1) * 128, :])
            for dc in range(ND):
                tp = tpsum.tile([128, 2, 128], f32, tag="tp", name=f"ktp{b}_{t}_{dc}")
                nc.tensor.transpose(tp[:, 0, :], ld[:, dc * 128:(dc + 1) * 128], ident_f32)
                nc.vector.tensor_copy(KT[:, dc, t, :], tp[:, 0, :])

        def prep_q(b, t):
            QT = batch_tiles[b]["QT"]
            ld = loads.tile([128, D], f32, tag="ld", name=f"qld{b}_{t}")
            nc.sync.dma_start(out=ld, in_=Qd[b, t * 128:(t + 1) * 128, :])
            nc.sync.dma_start(out=Rd[b, t * 128:(t + 1) * 128, D:2 * D], in_=ld)
            for dc in range(ND):
                tp = tpsum.tile([128, 2, 128], f32, tag="tp", name=f"qtp{b}_{t}_{dc}")
                nc.tensor.transpose(tp[:, 0, :], ld[:, dc * 128:(dc + 1) * 128], ident_f32)
                nc.vector.tensor_copy(QT[:, dc, t, :], tp[:, 0, :])

        def prep_v(b, t):
            Vb = batch_tiles[b]["Vb"]
            ld = loads.tile([128, D], f32, tag="vld", name=f"vld{b}_{t}")
            nc.sync.dma_start(out=ld, in_=Vd[b, t * 128:(t + 1) * 128, :])
            nc.vector.tensor_copy(Vb[:, t, :], ld)

        def flush_anorm():
            (b, qt, Ecs, rinv) = pending_anorm.pop(0)
            q0 = qt * 128
            for c in range(n_schunks):
                eng = anorm_engine
                if eng == "alternate":
                    eng = "scalar" if (qt * n_schunks + c) % 2 == 0 else "vector"
                if eng == "dve_bf16":
                    # normalize in bf16 on DVE (4x mode), cast to f32 in the
                    # SWDGE DMA (only gpsimd-issued DMAs may change dtype)
                    Ac = apool.tile([128, CW], bf16, tag="A", name=f"A{b}_{qt}_{c}")
                    nc.vector.tensor_scalar_mul(Ac, Ecs[c], rinv)
                    nc.gpsimd.dma_start(
                        out=Ad[b, q0:q0 + 128, c * CW:(c + 1) * CW], in_=Ac
                    )
                    continue
                Ac = apool.tile([128, CW], f32, tag="A", name=f"A{b}_{qt}_{c}")
                if eng == "scalar":
                    nc.scalar.activation(Ac, Ecs[c], AF.Copy, scale=rinv)
                else:
                    nc.vector.tensor_scalar_mul(Ac, Ecs[c], rinv)
                nc.sync.dma_start(
                    out=Ad[b, q0:q0 + 128, c * CW:(c + 1) * CW], in_=Ac
                )

        def main_tile(b, qt):
            bt = batch_tiles[b]
            KT, QT, Vb = bt["KT"], bt["QT"], bt["Vb"]
            q0 = qt * 128

            schunks = [
                spsum.tile([128, CW], f32, tag="schunk", name=f"schunk{b}_{qt}_{i}")
                for i in range(n_schunks)
            ]
            for dc in range(ND):
                for c in range(n_schunks):
                    for h in range(CW // 512):
                        ks = c * CW + h * 512
                        rhs = KT[:, dc, ks // 128:(ks + 512) // 128, :]
                        nc.tensor.matmul(
                            schunks[c][:, h * 512:(h + 1) * 512],
                            QT[:, dc, qt, :],
                            rhs,
                            start=(dc == 0),
                            stop=(dc == ND - 1),
                        )

            sumparts = small.tile([128, n_schunks], f32, tag="sp", name=f"sp{b}_{qt}")
            Ecs = []
            for c in range(n_schunks):
                Ec = epool.tile([128, CW], bf16, tag="E", name=f"E{b}_{qt}_{c}")
                Ecs.append(Ec)
                nc.scalar.activation(
                    Ec, schunks[c], AF.Exp,
                    scale=float(SCALE), accum_out=sumparts[:, c:c + 1],
                )

            # E^T per chunk (chunk 0 transposes overlap chunk 1 exp)
            ET = etpool.tile([128, NT, 128], bf16, tag="ET", name=f"ET{b}_{qt}")
            for c in range(n_schunks):
                for j in range(NC_T // 2):
                    tp = tpsum.tile([128, 2, 128], bf16, tag="tp", name=f"ttp{b}_{qt}_{c}_{j}")
                    for u in range(2):
                        kk = 2 * j + u
                        nc.tensor.transpose(
                            tp[:, u, :], Ecs[c][:, kk * 128:(kk + 1) * 128], ident_bf16
                        )
                    kc = c * NC_T + 2 * j
                    nc.vector.tensor_copy(ET[:, kc:kc + 2, :], tp)

            rinv = small.tile([128, 1], f32, tag="ri", name=f"ri{b}_{qt}")
            if n_schunks == 1:
                nc.vector.reciprocal(rinv, sumparts)
            else:
                rowsum = small.tile([128, 1], f32, tag="rs", name=f"rs{b}_{qt}")
                nc.vector.tensor_reduce(
                    out=rowsum, in_=sumparts,
                    axis=mybir.AxisListType.X, op=mybir.AluOpType.add,
                )
                nc.vector.reciprocal(rinv, rowsum)

            pending_anorm.append((b, qt, Ecs, rinv))
            if len(pending_anorm) > anorm_lag:
                flush_anorm()

            Opsum = opsum.tile([128, D], f32, tag="O", name=f"O{b}_{qt}")
            for k in range(NT):
                nc.tensor.matmul(
                    Opsum, ET[:, k, :], Vb[:, k, :],
                    start=(k == 0), stop=(k == NT - 1),
                )
            Rl = rpool.tile([128, D], f32, tag="Rl", name=f"Rl{b}_{qt}")
            nc.vector.tensor_scalar_mul(Rl, Opsum, rinv)
            nc.sync.dma_start(out=Rd[b, q0:q0 + 128, 0:D], in_=Rl)

        # ---- emission schedule ----
        prep_start(0)
        for t in range(NT):
            prep_k(0, t)
        prep_q(0, 0)
        for t in range(NT):
            prep_v(0, t)
        for b in range(BPC):
            for qt in range(NT):
                main_tile(b, qt)
                if qt + 1 < NT:
                    prep_q(b, qt + 1)
                if b + 1 < BPC:
                    if qt == 0:
                        prep_start(b + 1)
                    prep_k(b + 1, qt)
                    prep_v(b + 1, qt)
                    if qt == NT - 1:
                        prep_q(b + 1, 0)
        while pending_anorm:
            flush_anorm()

    nc.compile()
    return nc


_cached = {}


def _get_nc(**kw):
    key = tuple(sorted(kw.items()))
    if key not in _cached:
        _cached[key] = build_nc(**kw)
    return _cached[key]


def kernel(K, V, Q, **build_kw):
    from concourse.bass_utils import run_bass_kernel_spmd

    nc = _get_nc(**build_kw)
    K = np.asarray(K, dtype=np.float32)
    V = np.asarray(V, dtype=np.float32)
    Q = np.asarray(Q, dtype=np.float32)
    in_maps = [
        {
            "K": np.ascontiguousarray(K[c * BPC:(c + 1) * BPC]),
            "V": np.ascontiguousarray(V[c * BPC:(c + 1) * BPC]),
            "Q": np.ascontiguousarray(Q[c * BPC:(c + 1) * BPC]),
        }
        for c in range(NCORES)
    ]
    res = run_bass_kernel_spmd(nc, in_maps, core_ids=list(range(NCORES)))
    R = np.concatenate([res.results[c]["R"] for c in range(NCORES)], axis=0)
    A = np.concatenate([res.results[c]["A"] for c in range(NCORES)], axis=0)
    return (R, A)


# revision 12
# speedup vs baseline: 1.0851x; 1.0851x over previous
"""Batched attention kernel for Trainium2, SPMD over 8 NeuronCores.

Computes, for inputs K, V, Q of shape [16, 2048, 256] (f32):
    A = softmax(Q @ K^T / sqrt(256), axis=-1)      # [16, 2048, 2048]
    R = concat(A @ V, Q, axis=-1)                  # [16, 2048, 512]
and returns (R, A), matching the reference.

Sharding: batch dim across the 8 cores (2 batches per core), fully local.

Per-core dataflow (per batch):
  prep: load K/Q/V tiles f32; K^T/Q^T built via PE transpose + DVE
        evict-cast to bf16; V cast to bf16; Q f32 DMA'd out to R[..., D:].
  main (16 q-tiles of 128 rows):
        S = Q@K^T into PSUM (bf16 matmuls, f32 accum, 2 chunk tiles)
        E_c = exp(S_c/16) via ScalarE with free row-sum accumulation
        A = E * (1/rowsum) on ScalarE (deferred one tile so the next
        tile's exp isn't stuck behind it in the ScalarE queue), DMA out
        E^T via PE transposes + DVE evicts (per chunk, so chunk 0
        transposes overlap chunk 1's exp)
        O = E^T.T @ V accumulated in PSUM, scaled by 1/rowsum, DMA to R
  Softmax max-subtraction is skipped: scores ~ N(0,1), no overflow risk.
  Emission order staggers prep with compute: K(0) prep first, then Q/V(0)
  and the next batch's prep interleave into the running main loop.
"""

import numpy as np

B, T, D = 16, 2048, 256
NCORES = 8
BPC = B // NCORES   # batches per core
NT = T // 128       # 16 row-tiles per sequence
ND = D // 128       # 2 contraction chunks

SCALE = 1.0 / float(np.sqrt(np.float32(D)))  # 1/16


def build_nc(
    n_schunks=2,          # S psum chunks per q-tile (each T//n_schunks wide)
    spsum_bufs=2,
    tpsum_bufs=3,
    opsum_bufs=1,
    e_bufs=6,             # chunk-granular E tiles
    et_bufs=2,
    a_bufs=3,
    anorm_engine="scalar",  # "scalar" | "dve_bf16" | "vector" | "alternate"
    anorm_lag=1,          # tiles to defer the A-normalize + A DMA by
):
    from contextlib import ExitStack
    import concourse.bacc as bacc
    import concourse.tile as tile
    from concourse import mybir, masks

    f32 = mybir.dt.float32
    bf16 = mybir.dt.bfloat16
    AF = mybir.ActivationFunctionType

    CW = T // n_schunks            # chunk width in k
    NC_T = CW // 128               # k-subtiles per chunk
    assert CW % 512 == 0

    nc = bacc.Bacc(None, target_bir_lowering=False)
    Kd = nc.declare_dram_parameter("K", [BPC, T, D], f32, isOutput=False)
    Vd = nc.declare_dram_parameter("V", [BPC, T, D], f32, isOutput=False)
    Qd = nc.declare_dram_parameter("Q", [BPC, T, D], f32, isOutput=False)
    Rd = nc.declare_dram_parameter("R", [BPC, T, 2 * D], f32, isOutput=True)
    Ad = nc.declare_dram_parameter("A", [BPC, T, T], f32, isOutput=True)

    with tile.TileContext(nc) as tc, ExitStack() as ctx:
        singles = ctx.enter_context(tc.tile_pool(name="singles", bufs=1))
        batchp = ctx.enter_context(tc.tile_pool(name="batchp", bufs=2))
        loads = ctx.enter_context(tc.tile_pool(name="loads", bufs=6))
        epool = ctx.enter_context(tc.tile_pool(name="epool", bufs=e_bufs))
        etpool = ctx.enter_context(tc.tile_pool(name="etpool", bufs=et_bufs))
        apool = ctx.enter_context(tc.tile_pool(name="apool", bufs=a_bufs))
        rpool = ctx.enter_context(tc.tile_pool(name="rpool", bufs=3))
        small = ctx.enter_context(tc.tile_pool(name="small", bufs=8))
        spsum = ctx.enter_context(tc.tile_pool(name="spsum", bufs=spsum_bufs, space="PSUM"))
        opsum = ctx.enter_context(tc.tile_pool(name="opsum", bufs=opsum_bufs, space="PSUM"))
        tpsum = ctx.enter_context(tc.tile_pool(name="tpsum", bufs=tpsum_bufs, space="PSUM"))

        ident_f32 = singles.tile([128, 128], f32)
        masks.make_identity(nc, ident_f32)
        ident_bf16 = singles.tile([128, 128], bf16)
        masks.make_identity(nc, ident_bf16)

        batch_tiles = {}
        pending_anorm = []   # deferred (b, qt, Ecs, rinv) entries

        def prep_start(b):
            batch_tiles[b] = {
                "KT": batchp.tile([128, ND, NT, 128], bf16, tag="KT", name=f"KT{b}"),
                "QT": batchp.tile([128, ND, NT, 128], bf16, tag="QT", name=f"QT{b}"),
                "Vb": batchp.tile([128, NT, D], bf16, tag="Vb", name=f"Vb{b}"),
            }

        def prep_k(b, t):
            KT = batch_tiles[b]["KT"]
            ld = loads.tile([128, D], f32, tag="ld", name=f"kld{b}_{t}")
            nc.sync.dma_start(out=ld, in_=Kd[b, t * 128:(t + 1) * 128, :])
            for dc in range(ND):
                tp = tpsum.tile([128, 2, 128], f32, tag="tp", name=f"ktp{b}_{t}_{dc}")
                nc.tensor.transpose(tp[:, 0, :], ld[:, dc * 128:(dc + 1) * 128], ident_f32)
                nc.vector.tensor_copy(KT[:, dc, t, :], tp[:, 0, :])

        def prep_q(b, t):
            QT = batch_tiles[b]["QT"]
            ld = loads.tile([128, D], f32, tag="ld", name=f"qld{b}_{t}")
            nc.sync.dma_start(out=ld, in_=Qd[b, t * 128:(t + 1) * 128, :])
            nc.sync.dma_start(out=Rd[b, t * 128:(t + 1) * 128, D:2 * D], in_=ld)
            for dc in range(ND):
                tp = tpsum.tile([128, 2, 128], f32, tag="tp", name=f"qtp{b}_{t}_{dc}")
                nc.tensor.transpose(tp[:, 0, :], ld[:, dc * 128:(dc + 1) * 128], ident_f32)
                nc.vector.tensor_copy(QT[:, dc, t, :], tp[:, 0, :])

        def prep_v(b, t):
            Vb = batch_tiles[b]["Vb"]
            ld = loads.tile([128, D], f32, tag="vld", name=f"vld{b}_{t}")
            nc.sync.dma_start(out=ld, in_=Vd[b, t * 128:(t + 1) * 128, :])
            nc.vector.tensor_copy(Vb[:, t, :], ld)

        def flush_anorm():
            (b, qt, Ecs, rinv) = pending_anorm.pop(0)
            q0 = qt * 128
            for c in range(n_schunks):
                eng = anorm_engine
                if eng == "alternate":
                    eng = "scalar" if (qt * n_schunks + c) % 2 == 0 else "vector"
                if eng == "dve_bf16":
                    # normalize in bf16 on DVE (4x mode), cast to f32 in the
                    # SWDGE DMA (only gpsimd-issued DMAs may change dtype)
                    Ac = apool.tile([128, CW], bf16, tag="A", name=f"A{b}_{qt}_{c}")
                    nc.vector.tensor_scalar_mul(Ac, Ecs[c], rinv)
                    nc.gpsimd.dma_start(
                        out=Ad[b, q0:q0 + 128, c * CW:(c + 1) * CW], in_=Ac
                    )
                    continue
                Ac = apool.tile([128, CW], f32, tag="A", name=f"A{b}_{qt}_{c}")
                if eng == "scalar":
                    nc.scalar.activation(Ac, Ecs[c], AF.Copy, scale=rinv)
                else:
                    nc.vector.tensor_scalar_mul(Ac, Ecs[c], rinv)
                nc.sync.dma_start(
                    out=Ad[b, q0:q0 + 128, c * CW:(c + 1) * CW], in_=Ac
                )

        def main_tile(b, qt):
            bt = batch_tiles[b]
            KT, QT, Vb = bt["KT"], bt["QT"], bt["Vb"]
            q0 = qt * 128

            schunks = [
                spsum.tile([128, CW], f32, tag="schunk", name=f"schunk{b}_{qt}_{i}")
                for i in range(n_schunks)
            ]
            for dc in range(ND):
                for c in range(n_schunks):
                    for h in range(CW // 512):
                        ks = c * CW + h * 512
                        rhs = KT[:, dc, ks // 128:(ks + 512) // 128, :]
                        nc.tensor.matmul(
                            schunks[c][:, h * 512:(h + 1) * 512],
                            QT[:, dc, qt, :],
                            rhs,
                            start=(dc == 0),
                            stop=(dc == ND - 1),
                        )

            sumparts = small.tile([128, n_schunks], f32, tag="sp", name=f"sp{b}_{qt}")
            Ecs = []
            for c in range(n_schunks):
                Ec = epool.tile([128, CW], bf16, tag="E", name=f"E{b}_{qt}_{c}")
                Ecs.append(Ec)
                nc.scalar.activation(
                    Ec, schunks[c], AF.Exp,
                    scale=float(SCALE), accum_out=sumparts[:, c:c + 1],
                )

            # E^T per chunk (chunk 0 transposes overlap chunk 1 exp)
            ET = etpool.tile([128, NT, 128], bf16, tag="ET", name=f"ET{b}_{qt}")
            for c in range(n_schunks):
                for j in range(NC_T // 2):
                    tp = tpsum.tile([128, 2, 128], bf16, tag="tp", name=f"ttp{b}_{qt}_{c}_{j}")
                    for u in range(2):
                        kk = 2 * j + u
                        nc.tensor.transpose(
                            tp[:, u, :], Ecs[c][:, kk * 128:(kk + 1) * 128], ident_bf16
                        )
                    kc = c * NC_T + 2 * j
                    nc.vector.tensor_copy(ET[:, kc:kc + 2, :], tp)

            rinv = small.tile([128, 1], f32, tag="ri", name=f"ri{b}_{qt}")
            if n_schunks == 1:
                nc.vector.reciprocal(rinv, sumparts)
            else:
                rowsum = small.tile([128, 1], f32, tag="rs", name=f"rs{b}_{qt}")
                nc.vector.tensor_reduce(
                    out=rowsum, in_=sumparts,
                    axis=mybir.AxisListType.X, op=mybir.AluOpType.add,
                )
                nc.vector.reciprocal(rinv, rowsum)

            pending_anorm.append((b, qt, Ecs, rinv))
            if len(pending_anorm) > anorm_lag:
                flush_anorm()

            Opsum = opsum.tile([128, D], f32, tag="O", name=f"O{b}_{qt}")
            for k in range(NT):
                nc.tensor.matmul(
                    Opsum, ET[:, k, :], Vb[:, k, :],
                    start=(k == 0), stop=(k == NT - 1),
                )
            Rl = rpool.tile([128, D], f32, tag="Rl", name=f"Rl{b}_{qt}")
            nc.vector.tensor_scalar_mul(Rl, Opsum, rinv)
            nc.sync.dma_start(out=Rd[b, q0:q0 + 128, 0:D], in_=Rl)

        # ---- emission schedule ----
        prep_start(0)
        for t in range(NT):
            prep_k(0, t)
        prep_q(0, 0)
        for t in range(NT):
            prep_v(0, t)
        for b in range(BPC):
            for qt in range(NT):
                main_tile(b, qt)
                if qt + 1 < NT:
                    prep_q(b, qt + 1)
                if b + 1 < BPC:
                    if qt == 0:
                        prep_start(b + 1)
                    prep_k(b + 1, qt)
                    prep_v(b + 1, qt)
                    if qt == NT - 1:
                        prep_q(b + 1, 0)
        while pending_anorm:
            flush_anorm()

    nc.compile()
    return nc


_cached = {}


def _get_nc(**kw):
    key = tuple(sorted(kw.items()))
    if key not in _cached:
        _cached[key] = build_nc(**kw)
    return _cached[key]


def kernel(K, V, Q, **build_kw):
    from concourse.bass_utils import run_bass_kernel_spmd

    nc = _get_nc(**build_kw)
    K = np.asarray(K, dtype=np.float32)
    V = np.asarray(V, dtype=np.float32)
    Q = np.asarray(Q, dtype=np.float32)
    in_maps = [
        {
            "K": np.ascontiguousarray(K[c * BPC:(c + 1) * BPC]),
            "V": np.ascontiguousarray(V[c * BPC:(c + 1) * BPC]),
            "Q": np.ascontiguousarray(Q[c * BPC:(c + 1) * BPC]),
        }
        for c in range(NCORES)
    ]
    res = run_bass_kernel_spmd(nc, in_maps, core_ids=list(range(NCORES)))
    R = np.concatenate([res.results[c]["R"] for c in range(NCORES)], axis=0)
    A = np.concatenate([res.results[c]["A"] for c in range(NCORES)], axis=0)
    return (R, A)


# revision 13
# speedup vs baseline: 1.0892x; 1.0039x over previous
"""Batched attention kernel for Trainium2, SPMD over 8 NeuronCores.

Computes, for inputs K, V, Q of shape [16, 2048, 256] (f32):
    A = softmax(Q @ K^T / sqrt(256), axis=-1)      # [16, 2048, 2048]
    R = concat(A @ V, Q, axis=-1)                  # [16, 2048, 512]
and returns (R, A), matching the reference.

Sharding: batch dim across the 8 cores (2 batches per core), fully local.

Per-core dataflow (per batch):
  prep: load K/Q/V tiles f32; K^T/Q^T built via PE transpose + DVE
        evict-cast to bf16; V cast to bf16; Q f32 DMA'd out to R[..., D:].
  main (16 q-tiles of 128 rows):
        S = Q@K^T into PSUM (bf16 matmuls, f32 accum, 2 chunk tiles)
        E_c = exp(S_c/16) via ScalarE with free row-sum accumulation
        A = E * (1/rowsum) on ScalarE (deferred one tile so the next
        tile's exp isn't stuck behind it in the ScalarE queue), DMA out
        E^T via PE transposes + DVE evicts (per chunk, so chunk 0
        transposes overlap chunk 1's exp)
        O = E^T.T @ V accumulated in PSUM, scaled by 1/rowsum, DMA to R
  Softmax max-subtraction is skipped: scores ~ N(0,1), no overflow risk.
  Emission order staggers prep with compute: K(0) prep first, then Q/V(0)
  and the next batch's prep interleave into the running main loop.
"""

import numpy as np

B, T, D = 16, 2048, 256
NCORES = 8
BPC = B // NCORES   # batches per core
NT = T // 128       # 16 row-tiles per sequence
ND = D // 128       # 2 contraction chunks

SCALE = 1.0 / float(np.sqrt(np.float32(D)))  # 1/16


def build_nc(
    n_schunks=2,          # S psum chunks per q-tile (each T//n_schunks wide)
    spsum_bufs=2,
    tpsum_bufs=3,
    opsum_bufs=1,
    e_bufs=6,             # chunk-granular E tiles
    et_bufs=3,
    a_bufs=3,
    anorm_engine="scalar",  # "scalar" | "dve_bf16" | "vector" | "alternate"
    anorm_lag=1,          # tiles to defer the A-normalize + A DMA by
    o_lag=1,              # tiles to defer the O matmul phase by (decouples it
                          # from the same tile's E^T evict stream)
):
    from contextlib import ExitStack
    import concourse.bacc as bacc
    import concourse.tile as tile
    from concourse import mybir, masks

    f32 = mybir.dt.float32
    bf16 = mybir.dt.bfloat16
    AF = mybir.ActivationFunctionType

    CW = T // n_schunks            # chunk width in k
    NC_T = CW // 128               # k-subtiles per chunk
    assert CW % 512 == 0

    nc = bacc.Bacc(None, target_bir_lowering=False)
    Kd = nc.declare_dram_parameter("K", [BPC, T, D], f32, isOutput=False)
    Vd = nc.declare_dram_parameter("V", [BPC, T, D], f32, isOutput=False)
    Qd = nc.declare_dram_parameter("Q", [BPC, T, D], f32, isOutput=False)
    Rd = nc.declare_dram_parameter("R", [BPC, T, 2 * D], f32, isOutput=True)
    Ad = nc.declare_dram_parameter("A", [BPC, T, T], f32, isOutput=True)

    with tile.TileContext(nc) as tc, ExitStack() as ctx:
        singles = ctx.enter_context(tc.tile_pool(name="singles", bufs=1))
        batchp = ctx.enter_context(tc.tile_pool(name="batchp", bufs=2))
        loads = ctx.enter_context(tc.tile_pool(name="loads", bufs=6))
        epool = ctx.enter_context(tc.tile_pool(name="epool", bufs=e_bufs))
        etpool = ctx.enter_context(tc.tile_pool(name="etpool", bufs=et_bufs))
        apool = ctx.enter_context(tc.tile_pool(name="apool", bufs=a_bufs))
        rpool = ctx.enter_context(tc.tile_pool(name="rpool", bufs=3))
        small = ctx.enter_context(tc.tile_pool(name="small", bufs=8))
        spsum = ctx.enter_context(tc.tile_pool(name="spsum", bufs=spsum_bufs, space="PSUM"))
        opsum = ctx.enter_context(tc.tile_pool(name="opsum", bufs=opsum_bufs, space="PSUM"))
        tpsum = ctx.enter_context(tc.tile_pool(name="tpsum", bufs=tpsum_bufs, space="PSUM"))

        ident_f32 = singles.tile([128, 128], f32)
        masks.make_identity(nc, ident_f32)
        ident_bf16 = singles.tile([128, 128], bf16)
        masks.make_identity(nc, ident_bf16)

        batch_tiles = {}
        pending_anorm = []   # deferred (b, qt, Ecs, rinv) entries
        pending_o = []       # deferred (b, qt, ET, Vb, rinv) entries

        def flush_o():
            (b, qt, ET, Vb, rinv) = pending_o.pop(0)
            q0 = qt * 128
            Opsum = opsum.tile([128, D], f32, tag="O", name=f"O{b}_{qt}")
            for k in range(NT):
                nc.tensor.matmul(
                    Opsum, ET[:, k, :], Vb[:, k, :],
                    start=(k == 0), stop=(k == NT - 1),
                )
            Rl = rpool.tile([128, D], f32, tag="Rl", name=f"Rl{b}_{qt}")
            nc.vector.tensor_scalar_mul(Rl, Opsum, rinv)
            nc.sync.dma_start(out=Rd[b, q0:q0 + 128, 0:D], in_=Rl)

        def prep_start(b):
            batch_tiles[b] = {
                "KT": batchp.tile([128, ND, NT, 128], bf16, tag="KT", name=f"KT{b}"),
                "QT": batchp.tile([128, ND, NT, 128], bf16, tag="QT", name=f"QT{b}"),
                "Vb": batchp.tile([128, NT, D], bf16, tag="Vb", name=f"Vb{b}"),
            }

        def prep_k(b, t):
            KT = batch_tiles[b]["KT"]
            ld = loads.tile([128, D], f32, tag="ld", name=f"kld{b}_{t}")
            nc.sync.dma_start(out=ld, in_=Kd[b, t * 128:(t + 1) * 128, :])
            for dc in range(ND):
                tp = tpsum.tile([128, 2, 128], f32, tag="tp", name=f"ktp{b}_{t}_{dc}")
                nc.tensor.transpose(tp[:, 0, :], ld[:, dc * 128:(dc + 1) * 128], ident_f32)
                nc.vector.tensor_copy(KT[:, dc, t, :], tp[:, 0, :])

        def prep_q(b, t):
            QT = batch_tiles[b]["QT"]
            ld = loads.tile([128, D], f32, tag="ld", name=f"qld{b}_{t}")
            nc.sync.dma_start(out=ld, in_=Qd[b, t * 128:(t + 1) * 128, :])
            nc.sync.dma_start(out=Rd[b, t * 128:(t + 1) * 128, D:2 * D], in_=ld)
            for dc in range(ND):
                tp = tpsum.tile([128, 2, 128], f32, tag="tp", name=f"qtp{b}_{t}_{dc}")
                nc.tensor.transpose(tp[:, 0, :], ld[:, dc * 128:(dc + 1) * 128], ident_f32)
                nc.vector.tensor_copy(QT[:, dc, t, :], tp[:, 0, :])

        def prep_v(b, t):
            Vb = batch_tiles[b]["Vb"]
            ld = loads.tile([128, D], f32, tag="vld", name=f"vld{b}_{t}")
            nc.sync.dma_start(out=ld, in_=Vd[b, t * 128:(t + 1) * 128, :])
            nc.vector.tensor_copy(Vb[:, t, :], ld)

        def flush_anorm():
            (b, qt, Ecs, rinv) = pending_anorm.pop(0)
            q0 = qt * 128
            for c in range(n_schunks):
                eng = anorm_engine
                if eng == "alternate":
                    eng = "scalar" if (qt * n_schunks + c) % 2 == 0 else "vector"
                if eng == "dve_bf16":
                    # normalize in bf16 on DVE (4x mode), cast to f32 in the
                    # SWDGE DMA (only gpsimd-issued DMAs may change dtype)
                    Ac = apool.tile([128, CW], bf16, tag="A", name=f"A{b}_{qt}_{c}")
                    nc.vector.tensor_scalar_mul(Ac, Ecs[c], rinv)
                    nc.gpsimd.dma_start(
                        out=Ad[b, q0:q0 + 128, c * CW:(c + 1) * CW], in_=Ac
                    )
                    continue
                Ac = apool.tile([128, CW], f32, tag="A", name=f"A{b}_{qt}_{c}")
                if eng == "scalar":
                    nc.scalar.activation(Ac, Ecs[c], AF.Copy, scale=rinv)
                else:
                    nc.vector.tensor_scalar_mul(Ac, Ecs[c], rinv)
                nc.sync.dma_start(
                    out=Ad[b, q0:q0 + 128, c * CW:(c + 1) * CW], in_=Ac
                )

        def main_tile(b, qt):
            bt = batch_tiles[b]
            KT, QT, Vb = bt["KT"], bt["QT"], bt["Vb"]
            q0 = qt * 128

            schunks = [
                spsum.tile([128, CW], f32, tag="schunk", name=f"schunk{b}_{qt}_{i}")
                for i in range(n_schunks)
            ]
            for dc in range(ND):
                for c in range(n_schunks):
                    for h in range(CW // 512):
                        ks = c * CW + h * 512
                        rhs = KT[:, dc, ks // 128:(ks + 512) // 128, :]
                        nc.tensor.matmul(
                            schunks[c][:, h * 512:(h + 1) * 512],
                            QT[:, dc, qt, :],
                            rhs,
                            start=(dc == 0),
                            stop=(dc == ND - 1),
                        )

            if len(pending_o) >= o_lag:
                flush_o()

            sumparts = small.tile([128, n_schunks], f32, tag="sp", name=f"sp{b}_{qt}")
            Ecs = []
            for c in range(n_schunks):
                Ec = epool.tile([128, CW], bf16, tag="E", name=f"E{b}_{qt}_{c}")
                Ecs.append(Ec)
                nc.scalar.activation(
                    Ec, schunks[c], AF.Exp,
                    scale=float(SCALE), accum_out=sumparts[:, c:c + 1],
                )

            # E^T per chunk (chunk 0 transposes overlap chunk 1 exp)
            ET = etpool.tile([128, NT, 128], bf16, tag="ET", name=f"ET{b}_{qt}")
            for c in range(n_schunks):
                for j in range(NC_T // 2):
                    tp = tpsum.tile([128, 2, 128], bf16, tag="tp", name=f"ttp{b}_{qt}_{c}_{j}")
                    for u in range(2):
                        kk = 2 * j + u
                        nc.tensor.transpose(
                            tp[:, u, :], Ecs[c][:, kk * 128:(kk + 1) * 128], ident_bf16
                        )
                    kc = c * NC_T + 2 * j
                    nc.vector.tensor_copy(ET[:, kc:kc + 2, :], tp)

            rinv = small.tile([128, 1], f32, tag="ri", name=f"ri{b}_{qt}")
            if n_schunks == 1:
                nc.vector.reciprocal(rinv, sumparts)
            else:
                rowsum = small.tile([128, 1], f32, tag="rs", name=f"rs{b}_{qt}")
                nc.vector.tensor_reduce(
                    out=rowsum, in_=sumparts,
                    axis=mybir.AxisListType.X, op=mybir.AluOpType.add,
                )
                nc.vector.reciprocal(rinv, rowsum)

            pending_anorm.append((b, qt, Ecs, rinv))
            if len(pending_anorm) > anorm_lag:
                flush_anorm()

            pending_o.append((b, qt, ET, Vb, rinv))

        # ---- emission schedule ----
        prep_start(0)
        for t in range(NT):
            prep_k(0, t)
        prep_q(0, 0)
        for t in range(NT):
            prep_v(0, t)
        for b in range(BPC):
            for qt in range(NT):
                main_tile(b, qt)
                if qt + 1 < NT:
                    prep_q(b, qt + 1)
                if b + 1 < BPC:
                    if qt == 0:
                        prep_start(b + 1)
                    prep_k(b + 1, qt)
                    prep_v(b + 1, qt)
                    if qt == NT - 1:
                        prep_q(b + 1, 0)
        while pending_o:
            flush_o()
        while pending_anorm:
            flush_anorm()

    nc.compile()
    return nc


_cached = {}


def _get_nc(**kw):
    key = tuple(sorted(kw.items()))
    if key not in _cached:
        _cached[key] = build_nc(**kw)
    return _cached[key]


def kernel(K, V, Q, **build_kw):
    from concourse.bass_utils import run_bass_kernel_spmd

    nc = _get_nc(**build_kw)
    K = np.asarray(K, dtype=np.float32)
    V = np.asarray(V, dtype=np.float32)
    Q = np.asarray(Q, dtype=np.float32)
    in_maps = [
        {
            "K": np.ascontiguousarray(K[c * BPC:(c + 1) * BPC]),
            "V": np.ascontiguousarray(V[c * BPC:(c + 1) * BPC]),
            "Q": np.ascontiguousarray(Q[c * BPC:(c + 1) * BPC]),
        }
        for c in range(NCORES)
    ]
    res = run_bass_kernel_spmd(nc, in_maps, core_ids=list(range(NCORES)))
    R = np.concatenate([res.results[c]["R"] for c in range(NCORES)], axis=0)
    A = np.concatenate([res.results[c]["A"] for c in range(NCORES)], axis=0)
    return (R, A)


# revision 14
# speedup vs baseline: 1.1478x; 1.0538x over previous
"""Batched attention kernel for Trainium2, SPMD over 8 NeuronCores.

Computes, for inputs K, V, Q of shape [16, 2048, 256] (f32):
    A = softmax(Q @ K^T / sqrt(256), axis=-1)      # [16, 2048, 2048]
    R = concat(A @ V, Q, axis=-1)                  # [16, 2048, 512]
and returns (R, A), matching the reference.

Sharding: batch dim across the 8 cores (2 batches per core), fully local.

Per-core dataflow (per batch):
  prep: load K/Q/V tiles f32; K^T/Q^T built via PE transpose + DVE
        evict-cast to bf16; V cast to bf16; Q f32 DMA'd out to R[..., D:].
  main (16 q-tiles of 128 rows):
        S = Q@K^T into PSUM (bf16 matmuls, f32 accum, 2 chunk tiles)
        E_c = exp(S_c/16) via ScalarE with free row-sum accumulation
        A = E * (1/rowsum) on ScalarE (deferred one tile so the next
        tile's exp isn't stuck behind it in the ScalarE queue), DMA out
        E^T via PE transposes + DVE evicts (per chunk, so chunk 0
        transposes overlap chunk 1's exp)
        O = E^T.T @ V accumulated in PSUM, scaled by 1/rowsum, DMA to R
  Softmax max-subtraction is skipped: scores ~ N(0,1), no overflow risk.
  Emission order staggers prep with compute: K(0) prep first, then Q/V(0)
  and the next batch's prep interleave into the running main loop.
"""

import numpy as np

B, T, D = 16, 2048, 256
NCORES = 8
BPC = B // NCORES   # batches per core
NT = T // 128       # 16 row-tiles per sequence
ND = D // 128       # 2 contraction chunks

SCALE = 1.0 / float(np.sqrt(np.float32(D)))  # 1/16


def build_nc(
    n_schunks=2,          # S psum chunks per q-tile (each T//n_schunks wide)
    spsum_bufs=2,
    tpsum_bufs=3,
    opsum_bufs=1,
    e_bufs=6,             # chunk-granular E tiles
    et_bufs=3,
    a_bufs=3,
    anorm_engine="scalar",  # "scalar" | "dve_bf16" | "vector" | "alternate"
    anorm_lag=1,          # tiles to defer the A-normalize + A DMA by
    o_lag=1,              # tiles to defer the O matmul phase by (decouples it
                          # from the same tile's E^T evict stream)
    s_dc_inner=False,     # S matmul order: complete each 512-col group early
):
    from contextlib import ExitStack
    import concourse.bacc as bacc
    import concourse.tile as tile
    from concourse import mybir, masks

    f32 = mybir.dt.float32
    bf16 = mybir.dt.bfloat16
    AF = mybir.ActivationFunctionType

    CW = T // n_schunks            # chunk width in k
    NC_T = CW // 128               # k-subtiles per chunk
    assert CW % 512 == 0

    nc = bacc.Bacc(None, target_bir_lowering=False)
    Kd = nc.declare_dram_parameter("K", [BPC, T, D], f32, isOutput=False)
    Vd = nc.declare_dram_parameter("V", [BPC, T, D], f32, isOutput=False)
    Qd = nc.declare_dram_parameter("Q", [BPC, T, D], f32, isOutput=False)
    Rd = nc.declare_dram_parameter("R", [BPC, T, 2 * D], f32, isOutput=True)
    Ad = nc.declare_dram_parameter("A", [BPC, T, T], f32, isOutput=True)

    with tile.TileContext(nc) as tc, ExitStack() as ctx:
        singles = ctx.enter_context(tc.tile_pool(name="singles", bufs=1))
        batchp = ctx.enter_context(tc.tile_pool(name="batchp", bufs=2))
        loads = ctx.enter_context(tc.tile_pool(name="loads", bufs=6))
        epool = ctx.enter_context(tc.tile_pool(name="epool", bufs=e_bufs))
        etpool = ctx.enter_context(tc.tile_pool(name="etpool", bufs=et_bufs))
        apool = ctx.enter_context(tc.tile_pool(name="apool", bufs=a_bufs))
        rpool = ctx.enter_context(tc.tile_pool(name="rpool", bufs=3))
        small = ctx.enter_context(tc.tile_pool(name="small", bufs=8))
        spsum = ctx.enter_context(tc.tile_pool(name="spsum", bufs=spsum_bufs, space="PSUM"))
        opsum = ctx.enter_context(tc.tile_pool(name="opsum", bufs=opsum_bufs, space="PSUM"))
        tpsum = ctx.enter_context(tc.tile_pool(name="tpsum", bufs=tpsum_bufs, space="PSUM"))

        ident_f32 = singles.tile([128, 128], f32)
        masks.make_identity(nc, ident_f32)
        ident_bf16 = singles.tile([128, 128], bf16)
        masks.make_identity(nc, ident_bf16)

        batch_tiles = {}
        pending_anorm = []   # deferred (b, qt, Ecs, rinv) entries
        pending_o = []       # deferred (b, qt, ET, Vb, rinv) entries

        def flush_o():
            (b, qt, ET, Vb, rinv) = pending_o.pop(0)
            q0 = qt * 128
            Opsum = opsum.tile([128, D], f32, tag="O", name=f"O{b}_{qt}")
            for k in range(NT):
                nc.tensor.matmul(
                    Opsum, ET[:, k, :], Vb[:, k, :],
                    start=(k == 0), stop=(k == NT - 1),
                )
            Rl = rpool.tile([128, D], f32, tag="Rl", name=f"Rl{b}_{qt}")
            nc.vector.tensor_scalar_mul(Rl, Opsum, rinv)
            nc.sync.dma_start(out=Rd[b, q0:q0 + 128, 0:D], in_=Rl)

        def prep_start(b):
            batch_tiles[b] = {
                "KT": batchp.tile([128, ND, NT, 128], bf16, tag="KT", name=f"KT{b}"),
                "QT": batchp.tile([128, ND, NT, 128], bf16, tag="QT", name=f"QT{b}"),
                "Vb": batchp.tile([128, NT, D], bf16, tag="Vb", name=f"Vb{b}"),
            }

        def prep_k(b, t):
            KT = batch_tiles[b]["KT"]
            ld = loads.tile([128, D], f32, tag="ld", name=f"kld{b}_{t}")
            nc.sync.dma_start(out=ld, in_=Kd[b, t * 128:(t + 1) * 128, :])
            for dc in range(ND):
                tp = tpsum.tile([128, 2, 128], f32, tag="tp", name=f"ktp{b}_{t}_{dc}")
                nc.tensor.transpose(tp[:, 0, :], ld[:, dc * 128:(dc + 1) * 128], ident_f32)
                nc.vector.tensor_copy(KT[:, dc, t, :], tp[:, 0, :])

        def prep_q(b, t):
            QT = batch_tiles[b]["QT"]
            ld = loads.tile([128, D], f32, tag="ld", name=f"qld{b}_{t}")
            nc.sync.dma_start(out=ld, in_=Qd[b, t * 128:(t + 1) * 128, :])
            nc.sync.dma_start(out=Rd[b, t * 128:(t + 1) * 128, D:2 * D], in_=ld)
            for dc in range(ND):
                tp = tpsum.tile([128, 2, 128], f32, tag="tp", name=f"qtp{b}_{t}_{dc}")
                nc.tensor.transpose(tp[:, 0, :], ld[:, dc * 128:(dc + 1) * 128], ident_f32)
                nc.vector.tensor_copy(QT[:, dc, t, :], tp[:, 0, :])

        def prep_v(b, t):
            Vb = batch_tiles[b]["Vb"]
            ld = loads.tile([128, D], f32, tag="vld", name=f"vld{b}_{t}")
            nc.sync.dma_start(out=ld, in_=Vd[b, t * 128:(t + 1) * 128, :])
            nc.vector.tensor_copy(Vb[:, t, :], ld)

        def flush_anorm():
            (b, qt, Ecs, rinv) = pending_anorm.pop(0)
            q0 = qt * 128
            for c in range(n_schunks):
                eng = anorm_engine
                if eng == "alternate":
                    eng = "scalar" if (qt * n_schunks + c) % 2 == 0 else "vector"
                if eng == "dve_bf16":
                    # normalize in bf16 on DVE (4x mode), cast to f32 in the
                    # SWDGE DMA (only gpsimd-issued DMAs may change dtype)
                    Ac = apool.tile([128, CW], bf16, tag="A", name=f"A{b}_{qt}_{c}")
                    nc.vector.tensor_scalar_mul(Ac, Ecs[c], rinv)
                    nc.gpsimd.dma_start(
                        out=Ad[b, q0:q0 + 128, c * CW:(c + 1) * CW], in_=Ac
                    )
                    continue
                Ac = apool.tile([128, CW], f32, tag="A", name=f"A{b}_{qt}_{c}")
                if eng == "scalar":
                    nc.scalar.activation(Ac, Ecs[c], AF.Copy, scale=rinv)
                else:
                    nc.vector.tensor_scalar_mul(Ac, Ecs[c], rinv)
                nc.sync.dma_start(
                    out=Ad[b, q0:q0 + 128, c * CW:(c + 1) * CW], in_=Ac
                )

        def main_tile(b, qt):
            bt = batch_tiles[b]
            KT, QT, Vb = bt["KT"], bt["QT"], bt["Vb"]
            q0 = qt * 128

            schunks = [
                spsum.tile([128, CW], f32, tag="schunk", name=f"schunk{b}_{qt}_{i}")
                for i in range(n_schunks)
            ]
            if s_dc_inner:
                # finish each 512-wide group (all dc) before moving on, so
                # chunk 0 is exp-ready after its own matmuls, not the sweep
                order = [
                    (dc, c, h)
                    for c in range(n_schunks)
                    for h in range(CW // 512)
                    for dc in range(ND)
                ]
            else:
                order = [
                    (dc, c, h)
                    for dc in range(ND)
                    for c in range(n_schunks)
                    for h in range(CW // 512)
                ]
            for (dc, c, h) in order:
                ks = c * CW + h * 512
                rhs = KT[:, dc, ks // 128:(ks + 512) // 128, :]
                nc.tensor.matmul(
                    schunks[c][:, h * 512:(h + 1) * 512],
                    QT[:, dc, qt, :],
                    rhs,
                    start=(dc == 0),
                    stop=(dc == ND - 1),
                )

            if len(pending_o) >= o_lag:
                flush_o()

            sumparts = small.tile([128, n_schunks], f32, tag="sp", name=f"sp{b}_{qt}")
            Ecs = []
            for c in range(n_schunks):
                Ec = epool.tile([128, CW], bf16, tag="E", name=f"E{b}_{qt}_{c}")
                Ecs.append(Ec)
                nc.scalar.activation(
                    Ec, schunks[c], AF.Exp,
                    scale=float(SCALE), accum_out=sumparts[:, c:c + 1],
                )

            # E^T per chunk (chunk 0 transposes overlap chunk 1 exp)
            ET = etpool.tile([128, NT, 128], bf16, tag="ET", name=f"ET{b}_{qt}")
            for c in range(n_schunks):
                for j in range(NC_T // 2):
                    tp = tpsum.tile([128, 2, 128], bf16, tag="tp", name=f"ttp{b}_{qt}_{c}_{j}")
                    for u in range(2):
                        kk = 2 * j + u
                        nc.tensor.transpose(
                            tp[:, u, :], Ecs[c][:, kk * 128:(kk + 1) * 128], ident_bf16
                        )
                    kc = c * NC_T + 2 * j
                    nc.vector.tensor_copy(ET[:, kc:kc + 2, :], tp)

            rinv = small.tile([128, 1], f32, tag="ri", name=f"ri{b}_{qt}")
            if n_schunks == 1:
                nc.vector.reciprocal(rinv, sumparts)
            else:
                rowsum = small.tile([128, 1], f32, tag="rs", name=f"rs{b}_{qt}")
                nc.vector.tensor_reduce(
                    out=rowsum, in_=sumparts,
                    axis=mybir.AxisListType.X, op=mybir.AluOpType.add,
                )
                nc.vector.reciprocal(rinv, rowsum)

            pending_anorm.append((b, qt, Ecs, rinv))
            if len(pending_anorm) > anorm_lag:
                flush_anorm()

            pending_o.append((b, qt, ET, Vb, rinv))

        # ---- emission schedule ----
        prep_start(0)
        for t in range(NT):
            prep_k(0, t)
        prep_q(0, 0)
        for t in range(NT):
            prep_v(0, t)
        for b in range(BPC):
            for qt in range(NT):
                main_tile(b, qt)
                if qt + 1 < NT:
                    prep_q(b, qt + 1)
                if b + 1 < BPC:
                    if qt == 0:
                        prep_start(b + 1)
                    prep_k(b + 1, qt)
                    prep_v(b + 1, qt)
                    if qt == NT - 1:
                        prep_q(b + 1, 0)
        while pending_o:
            flush_o()
        while pending_anorm:
            flush_anorm()

    nc.compile()
    return nc


_cached = {}


def _get_nc(**kw):
    key = tuple(sorted(kw.items()))
    if key not in _cached:
        _cached[key] = build_nc(**kw)
    return _cached[key]


def kernel(K, V, Q, **build_kw):
    from concourse.bass_utils import run_bass_kernel_spmd

    nc = _get_nc(**build_kw)
    K = np.asarray(K, dtype=np.float32)
    V = np.asarray(V, dtype=np.float32)
    Q = np.asarray(Q, dtype=np.float32)
    in_maps = [
        {
            "K": np.ascontiguousarray(K[c * BPC:(c + 1) * BPC]),
            "V": np.ascontiguousarray(V[c * BPC:(c + 1) * BPC]),
            "Q": np.ascontiguousarray(Q[c * BPC:(c + 1) * BPC]),
        }
        for c in range(NCORES)
    ]
    res = run_bass_kernel_spmd(nc, in_maps, core_ids=list(range(NCORES)))
    R = np.concatenate([res.results[c]["R"] for c in range(NCORES)], axis=0)
    A = np.concatenate([res.results[c]["A"] for c in range(NCORES)], axis=0)
    return (R, A)
